# revision 57
# baseline (speedup 1.0000x reference)
"""MoE classifier (B=8192, DIN=1024, HID=4096, C=1000, E=8, K=2) on 8 TRN2
NeuronCores via expert parallelism.

Sharding strategy (host side = sharding/unsharding only):
  - Gating runs on host in float64 (it decides where tokens go — this IS the
    all-to-all routing step of the expert-parallel sharding).
  - Core e receives the tokens whose top-2 experts include e, pre-transposed
    to [DIN, cap] plus that expert's weights; it computes
    comb_t * (relu(x W1[e] + b1[e]) W2[e] + b2[e]) for its tokens.
  - Host scatter-adds the per-expert partial rows into the [B, C] output
    (ascending expert order, matching the reference einsum's accumulation).

Device kernel (per core, SPMD — no collectives):
  layer 1: hT[j*128+p, t] tiles = relu(W1_chunk.T @ xT_chunk + b1) via PSUM
           accumulation over 8 DIN chunks; bias+relu fused at PSUM eviction
           on ScalarE.
  layer 2: out[t, c] tiles = hT_tiles.T @ W2_chunks accumulated over 32 HID
           chunks; at PSUM eviction DVE adds b2 (host-replicated [128, C]
           tile) and multiplies by the per-token comb weight (per-partition
           scalar).
Matmuls run in float32r (TF32-like, full PE rate for N>=256) with fp32 PSUM
accumulation; weights stream from HBM (no SBUF residency needed at 4B width).
"""

import numpy as np
import ml_dtypes

import concourse.bass as bass
import concourse.bacc as bacc
import concourse.mybir as mybir
import concourse.tile as tile
from concourse.bass_utils import run_bass_kernel_spmd

B, DIN, HID, C, E, TOPK = 8192, 1024, 4096, 1000, 8, 2
P = 128
NCORES = 8
F32 = mybir.dt.float32
F32R = mybir.dt.float32r
BF16 = mybir.dt.bfloat16


def _np_dt(dt):
    return ml_dtypes.bfloat16 if dt == BF16 else np.float32


def route_host(x, Wg, bg):
    """Float64 gating + top-2 routing. Returns per-expert token indices,
    renormalized combine weights, and the load-balancing loss."""
    logits = x.astype(np.float64) @ Wg.astype(np.float64) + bg.astype(np.float64)
    logits -= logits.max(axis=1, keepdims=True)
    probs = np.exp(logits)
    probs /= probs.sum(axis=1, keepdims=True)

    # Descending top-2, ties -> lower index (matches jax.lax.top_k).
    order = np.argsort(-probs, axis=1, kind="stable")[:, :TOPK]  # [B, 2]
    tvals = np.take_along_axis(probs, order, axis=1)             # [B, 2]
    tvals = tvals / tvals.sum(axis=1, keepdims=True)

    idx_per_e, w_per_e = [], []
    for e in range(E):
        tok, slot = np.nonzero(order == e)
        idx_per_e.append(tok.astype(np.int64))
        w_per_e.append(tvals[tok, slot].astype(np.float32))

    lb_loss = np.float32(E * np.sum(probs.mean(axis=0) ** 2))
    return idx_per_e, w_per_e, lb_loss, probs, order


def build_program(cap, din=DIN, hid=HID, c_dim=C, tok_blk=768, dt_mm=F32R,
                  stream_w=True, repeats=1, w1_bufs=3, w2_bufs=4, x_bufs=2,
                  ps1_bufs=2):
    """One SPMD program; per-core data differs only through in_maps.

    stream_w: stream W1/W2 chunks from HBM per token block (needed at 4-byte
    dtypes; at bf16 both weight matrices are kept SBUF-resident instead).
    """
    nc = bacc.Bacc("TRN2", target_bir_lowering=False, debug=False,
                   num_devices=NCORES)
    nk1 = din // P        # DIN chunks (contraction, layer 1)
    nk2 = hid // P        # HID chunks (contraction, layer 2)
    ncolt = (cap + P - 1) // P

    xT = nc.dram_tensor("xT", [din, cap], dt_mm, kind="ExternalInput")
    if stream_w:
        # W1 pre-tiled on host: w1p[j, ki, ko, n] = W1[ko*P+ki, j*P+n], so
        # one column block [din, P] is a single contiguous 512 KiB read.
        w1 = nc.dram_tensor("w1", [nk2, P, nk1, P], dt_mm,
                            kind="ExternalInput")
    else:
        w1 = nc.dram_tensor("w1", [din, hid], dt_mm, kind="ExternalInput")
    b1c = nc.dram_tensor("b1c", [P, nk2], F32, kind="ExternalInput")
    w2 = nc.dram_tensor("w2", [hid, c_dim], dt_mm, kind="ExternalInput")
    # b2 replicated across partitions on host: added at PSUM eviction on DVE
    # (cheaper than a K=1 bias matmul on PE, and exact fp32 — no f32r
    # truncation of the bias).
    b2rep = nc.dram_tensor("b2rep", [P, c_dim], F32, kind="ExternalInput")
    comb = nc.dram_tensor("comb", [P, ncolt], F32, kind="ExternalInput")
    out = nc.dram_tensor("out", [cap, c_dim], F32, kind="ExternalOutput")

    # A ragged tail of < ~30 tokens past the last full 128-token tile would
    # still stream full 32x1000-row L2 tiles (+5% PE here). Those tokens run
    # class-major with swapped operands (W2 stationary, hT tail moving ->
    # `tail` rows per matmul) inside the main c0/j loop, reusing the live W2
    # slice and the PSUM bank the shortened main loop frees; bias+comb for
    # them are applied on the host.
    tail = cap % P
    tail = tail if 0 < tail <= 30 else 0
    tail_out = None
    if tail:
        tail_out = nc.dram_tensor("tail_out", [c_dim, tail], F32,
                                  kind="ExternalOutput")

    with tile.TileContext(nc) as tc:
        with (
            tc.tile_pool(name="wpool", bufs=1) as wpool,
            tc.tile_pool(name="w1s", bufs=w1_bufs) as w1s,
            tc.tile_pool(name="w2s", bufs=w2_bufs) as w2s,
            tc.tile_pool(name="cpool", bufs=1) as cpool,
            tc.tile_pool(name="xpool", bufs=x_bufs) as xpool,
            tc.tile_pool(name="hpool", bufs=1) as hpool,
            tc.tile_pool(name="opool", bufs=4) as opool,
            tc.tile_pool(name="ps1", bufs=ps1_bufs, space="PSUM") as ps1,
            tc.tile_pool(name="ps2", bufs=1, space="PSUM") as ps2,
        ):
            # ---- resident weights (bf16 only) / constants ----
            if not stream_w:
                w1_sb, w2_sb = [], []
                for k in range(nk1):
                    t = wpool.tile([P, hid], dt_mm, tag=f"w1_{k}",
                                   name=f"w1sb{k}")
                    nc.sync.dma_start(t[:], w1[k * P:(k + 1) * P, :])
                    w1_sb.append(t)
                for k in range(nk2):
                    t = wpool.tile([P, c_dim], dt_mm, tag=f"w2_{k}",
                                   name=f"w2sb{k}")
                    nc.sync.dma_start(t[:], w2[k * P:(k + 1) * P, :])
                    w2_sb.append(t)
            b1_sb = cpool.tile([P, nk2], F32, tag="b1")
            nc.sync.dma_start(b1_sb[:], b1c[:])
            comb_sb = cpool.tile([P, ncolt], F32, tag="comb")
            nc.sync.dma_start(comb_sb[:], comb[:])
            b2_sb = cpool.tile([P, c_dim], F32, tag="b2")
            nc.sync.dma_start(b2_sb[:], b2rep[:])

            for _ in range(repeats):
                for t0 in range(0, cap, tok_blk):
                    tb = min(tok_blk, cap - t0)
                    # ---- stream this token block (transposed) ----
                    # The j=0 weight block is issued before the xT tiles so
                    # the first matmul is not queued behind the whole block.
                    w1_first = None
                    if stream_w:
                        w1_first = w1s.tile([P, nk1, P], dt_mm, tag="w1s",
                                            name="w1t")
                        nc.sync.dma_start(w1_first[:], w1[0])
                    xt = []
                    for k in range(nk1):
                        t = xpool.tile([P, tok_blk], dt_mm, tag=f"xt_{k}",
                                       name=f"xt{k}")
                        nc.sync.dma_start(
                            t[:, :tb], xT[k * P:(k + 1) * P, t0:t0 + tb])
                        xt.append(t)
                    # ---- layer 1: hT tiles [128 hid, tb tok] ----
                    # PSUM tiles are <=512 f32 wide, so each j group runs in
                    # sub-blocks sharing one W1 column-block load. Sub sizes
                    # are balanced (float32r matmuls drop to 1/4 rate below
                    # 256 moving rows, so avoid e.g. (512, 134) tails) and
                    # even (fp32r ISA requires even moving/dst sizes).
                    n_sub = max(1, -(-tb // 512))
                    base2, extra2 = divmod(tb // 2, n_sub)
                    subs, s0 = [], 0
                    for i in range(n_sub):
                        sw = 2 * (base2 + (1 if i < extra2 else 0))
                        if i == n_sub - 1:
                            sw = tb - s0
                        subs.append((s0, sw))
                        s0 += sw
                    ht = []
                    for j in range(nk2):
                        if stream_w:
                            if j == 0:
                                w1t = w1_first
                            else:
                                w1t = w1s.tile([P, nk1, P], dt_mm, tag="w1s",
                                               name="w1t")
                                nc.sync.dma_start(w1t[:], w1[j])
                        h = hpool.tile([P, tok_blk], dt_mm, tag=f"ht_{j}",
                                       name=f"ht{j}")
                        for s0, sw in subs:
                            ps = ps1.tile([P, 512], F32, tag="ps1",
                                          name="ps1t")
                            for k in range(nk1):
                                lhsT = (w1t[:, k, :] if stream_w
                                        else w1_sb[k][:, j * P:(j + 1) * P])
                                nc.tensor.matmul(
                                    ps[:, :sw], lhsT, xt[k][:, s0:s0 + sw],
                                    start=(k == 0), stop=(k == nk1 - 1))
                            nc.scalar.activation(
                                h[:, s0:s0 + sw], ps[:, :sw],
                                mybir.ActivationFunctionType.Relu,
                                bias=b1_sb[:, j:j + 1])
                        ht.append(h)
                    # ---- layer 2: out tiles [tw tok, cw classes] ----
                    # c0 outer + j middle keeps only ceil(tb/P) PSUM groups
                    # live while W2 streams through a small slice pool.
                    blk_tail = tail if t0 + tb == cap else 0
                    tb_main = tb - blk_tail
                    n_ts = (tb_main + P - 1) // P
                    tail_stride = 8   # 32 B chunk stride in PSUM (8B align)
                    for c0 in range(0, c_dim, 512):
                        cw = min(512, c_dim - c0)
                        pss = [ps2.tile([P, 512], F32, tag=f"ps2_{i}",
                                        name=f"ps2t{i}") for i in range(n_ts)]
                        ps_tail = None
                        if blk_tail:
                            # class-major tail accumulator in the PSUM bank
                            # the shortened main loop leaves free
                            ps_tail = ps2.tile([P, 512], F32,
                                               tag=f"ps2_{n_ts}",
                                               name="ps2tail")
                            n_cc = (cw + P - 1) // P
                            # chunk order per j; at the closing j a full-128-
                            # row chunk goes last so every eviction can hang
                            # its dependency on the closing matmul's columns
                            cc_full = max(
                                c for c in range(n_cc)
                                if min(P, cw - c * P) == P)
                            cc_close = [c for c in range(n_cc)
                                        if c != cc_full] + [cc_full]
                        for j in range(nk2):
                            if stream_w:
                                w2j = w2s.tile([P, 512], dt_mm, tag="w2s",
                                               name="w2t")
                                nc.sync.dma_start(
                                    w2j[:, :cw],
                                    w2[j * P:(j + 1) * P, c0:c0 + cw])
                                w2j = w2j[:, :cw]
                            else:
                                w2j = w2_sb[j][:, c0:c0 + cw]
                            for i in range(n_ts):
                                ts = i * P
                                tw = min(P, tb - ts)
                                nc.tensor.matmul(
                                    pss[i][:tw, :cw],
                                    ht[j][:, ts:ts + tw],
                                    w2j,
                                    start=(j == 0), stop=(j == nk2 - 1))
                            if blk_tail:
                                # one PSUM group spans the whole bank: start
                                # clears the zero region once, later chunks
                                # overwrite-then-accumulate via has_written
                                order = (cc_close if j == nk2 - 1
                                         else range(n_cc))
                                for oi, cc in enumerate(order):
                                    ccw = min(P, cw - cc * P)
                                    mm = nc.tensor.matmul(
                                        ps_tail[:ccw,
                                                cc * tail_stride:
                                                cc * tail_stride + blk_tail],
                                        w2j[:, cc * P:cc * P + ccw],
                                        ht[j][:, tb_main:tb_main + blk_tail],
                                        start=(j == 0 and oi == 0),
                                        stop=(j == nk2 - 1 and
                                              oi == n_cc - 1))
                        for i in range(n_ts):
                            ts = i * P
                            tw = min(P, tb - ts)
                            col = (t0 + ts) // P
                            ot = opool.tile([P, 512], F32, tag="ot", name="ot")
                            nc.vector.tensor_tensor(
                                ot[:tw, :cw], pss[i][:tw, :cw],
                                b2_sb[:tw, c0:c0 + cw],
                                op=mybir.AluOpType.add)
                            nc.vector.tensor_scalar_mul(
                                ot[:tw, :cw], ot[:tw, :cw],
                                comb_sb[:tw, col:col + 1])
                            nc.sync.dma_start(
                                out[t0 + ts:t0 + ts + tw, c0:c0 + cw],
                                ot[:tw, :cw])
                        if blk_tail:
                            # evict the group-closing chunk first (its read
                            # waits for the closing matmul); the other
                            # chunks read that SBUF copy as a bypass in1 so
                            # they are transitively ordered after the close
                            # (tensor_tensor may read only one PSUM input)
                            cl0 = cc_full * tail_stride
                            ot_cl = opool.tile([P, 512], F32, tag="ot",
                                               name="ot")
                            nc.vector.tensor_copy(
                                ot_cl[:, :blk_tail],
                                ps_tail[:, cl0:cl0 + blk_tail])
                            r0 = c0 + cc_full * P
                            nc.sync.dma_start(
                                tail_out[r0:r0 + P, :],
                                ot_cl[:, :blk_tail])
                            for cc in range(n_cc):
                                if cc == cc_full:
                                    continue
                                ccw = min(P, cw - cc * P)
                                ot = opool.tile([P, 512], F32, tag="ot",
                                                name="ot")
                                nc.vector.tensor_tensor(
                                    ot[:ccw, :blk_tail],
                                    ps_tail[:ccw,
                                            cc * tail_stride:
                                            cc * tail_stride + blk_tail],
                                    ot_cl[:ccw, :blk_tail],
                                    op=mybir.AluOpType.bypass)
                                r0 = c0 + cc * P
                                nc.sync.dma_start(
                                    tail_out[r0:r0 + ccw, :],
                                    ot[:ccw, :blk_tail])
    nc.compile()
    return nc


def make_in_maps(x, W1, b1, W2, b2, idx_per_e, w_per_e, cap, dt_mm=F32R,
                 stream_w=True):
    """Per-core input dicts (this is the scatter half of the all-to-all)."""
    npdt = _np_dt(dt_mm)
    nk1, nk2 = DIN // P, HID // P
    ncolt = (cap + P - 1) // P
    in_maps = []
    for e in range(E):
        idx, wts = idx_per_e[e], w_per_e[e]
        n = len(idx)
        xT = np.zeros((DIN, cap), dtype=npdt)
        xT[:, :n] = x[idx].T.astype(npdt)
        flat = np.zeros(ncolt * P, dtype=np.float32)
        flat[:n] = wts
        combp = np.ascontiguousarray(flat.reshape(ncolt, P).T)
        w1e = W1[e].astype(npdt)
        if stream_w:
            w1e = np.ascontiguousarray(
                w1e.reshape(nk1, P, nk2, P).transpose(2, 1, 0, 3))
        in_maps.append({
            "xT": xT,
            "w1": w1e,
            "b1c": np.ascontiguousarray(b1[e].reshape(HID // P, P).T),
            "w2": W2[e].astype(npdt),
            "b2rep": np.ascontiguousarray(
                np.broadcast_to(b2[e].astype(np.float32), (P, C))),
            "comb": combp,
        })
    return in_maps


def combine_host(results, idx_per_e, w_per_e, b2, cap):
    tail = cap % P
    tail = tail if 0 < tail <= 30 else 0
    n_main_cap = cap - tail
    out = np.zeros((B, C), dtype=np.float32)
    for e in range(E):
        idx, wts = idx_per_e[e], w_per_e[e]
        n = len(idx)
        n_main = min(n, n_main_cap)
        if n_main:
            # idx is unique within one expert (an expert appears at most once
            # in a token's top-2), so fancy-index += is a safe scatter-add.
            out[idx[:n_main]] += results[e]["out"][:n_main]
        if n > n_main:
            # tokens from the swapped-operand tail path: raw h@W2 came back
            # class-major; apply bias and comb weight here.
            k = n - n_main
            y = results[e]["tail_out"][:, :k].T + b2[e]
            out[idx[n_main:n]] += wts[n_main:n, None] * y
    return out


def kernel(x, Wg, bg, W1, b1, W2, b2):
    x = np.asarray(x); Wg = np.asarray(Wg); bg = np.asarray(bg)
    W1 = np.asarray(W1); b1 = np.asarray(b1)
    W2 = np.asarray(W2); b2 = np.asarray(b2)

    idx_per_e, w_per_e, lb_loss, _, _ = route_host(x, Wg, bg)
    # fp32r ISA needs even matmul sizes; keep cap even so every sub-block is.
    cap = max(P, max(len(i) for i in idx_per_e))
    cap += cap % 2

    nc = build_program(cap, w1_bufs=4, w2_bufs=8)
    in_maps = make_in_maps(x, W1, b1, W2, b2, idx_per_e, w_per_e, cap)
    res = run_bass_kernel_spmd(nc, in_maps, list(range(NCORES)))
    out = combine_host(res.results, idx_per_e, w_per_e, b2, cap)
    return out, lb_loss


# revision 58
# speedup vs baseline: 1.0053x; 1.0053x over previous
"""MoE classifier (B=8192, DIN=1024, HID=4096, C=1000, E=8, K=2) on 8 TRN2
NeuronCores via expert parallelism.

Sharding strategy (host side = sharding/unsharding only):
  - Gating runs on host in float64 (it decides where tokens go — this IS the
    all-to-all routing step of the expert-parallel sharding).
  - Core e receives the tokens whose top-2 experts include e, pre-transposed
    to [DIN, cap] plus that expert's weights; it computes
    comb_t * (relu(x W1[e] + b1[e]) W2[e] + b2[e]) for its tokens.
  - Host scatter-adds the per-expert partial rows into the [B, C] output
    (ascending expert order, matching the reference einsum's accumulation).

Device kernel (per core, SPMD — no collectives):
  layer 1: hT[j*128+p, t] tiles = relu(W1_chunk.T @ xT_chunk + b1) via PSUM
           accumulation over 8 DIN chunks; bias+relu fused at PSUM eviction
           on ScalarE.
  layer 2: out[t, c] tiles = hT_tiles.T @ W2_chunks accumulated over 32 HID
           chunks; at PSUM eviction DVE adds b2 (host-replicated [128, C]
           tile) and multiplies by the per-token comb weight (per-partition
           scalar).
Matmuls run in float32r (TF32-like, full PE rate for N>=256) with fp32 PSUM
accumulation; weights stream from HBM (no SBUF residency needed at 4B width).
"""

import numpy as np
import ml_dtypes

import concourse.bass as bass
import concourse.bacc as bacc
import concourse.mybir as mybir
import concourse.tile as tile
from concourse.bass_utils import run_bass_kernel_spmd

B, DIN, HID, C, E, TOPK = 8192, 1024, 4096, 1000, 8, 2
P = 128
NCORES = 8
F32 = mybir.dt.float32
F32R = mybir.dt.float32r
BF16 = mybir.dt.bfloat16


def _np_dt(dt):
    return ml_dtypes.bfloat16 if dt == BF16 else np.float32


def route_host(x, Wg, bg):
    """Float64 gating + top-2 routing. Returns per-expert token indices,
    renormalized combine weights, and the load-balancing loss."""
    logits = x.astype(np.float64) @ Wg.astype(np.float64) + bg.astype(np.float64)
    logits -= logits.max(axis=1, keepdims=True)
    probs = np.exp(logits)
    probs /= probs.sum(axis=1, keepdims=True)

    # Descending top-2, ties -> lower index (matches jax.lax.top_k).
    order = np.argsort(-probs, axis=1, kind="stable")[:, :TOPK]  # [B, 2]
    tvals = np.take_along_axis(probs, order, axis=1)             # [B, 2]
    tvals = tvals / tvals.sum(axis=1, keepdims=True)

    idx_per_e, w_per_e = [], []
    for e in range(E):
        tok, slot = np.nonzero(order == e)
        idx_per_e.append(tok.astype(np.int64))
        w_per_e.append(tvals[tok, slot].astype(np.float32))

    lb_loss = np.float32(E * np.sum(probs.mean(axis=0) ** 2))
    return idx_per_e, w_per_e, lb_loss, probs, order


def build_program(cap, din=DIN, hid=HID, c_dim=C, tok_blk=768, dt_mm=F32R,
                  stream_w=True, repeats=1, w1_bufs=3, w2_bufs=4, x_bufs=2,
                  ps1_bufs=2):
    """One SPMD program; per-core data differs only through in_maps.

    stream_w: stream W1/W2 chunks from HBM per token block (needed at 4-byte
    dtypes; at bf16 both weight matrices are kept SBUF-resident instead).
    """
    nc = bacc.Bacc("TRN2", target_bir_lowering=False, debug=False,
                   num_devices=NCORES)
    nk1 = din // P        # DIN chunks (contraction, layer 1)
    nk2 = hid // P        # HID chunks (contraction, layer 2)
    ncolt = (cap + P - 1) // P

    xT = nc.dram_tensor("xT", [din, cap], dt_mm, kind="ExternalInput")
    if stream_w:
        # W1 pre-tiled on host: w1p[j, ki, ko, n] = W1[ko*P+ki, j*P+n], so
        # one column block [din, P] is a single contiguous 512 KiB read.
        w1 = nc.dram_tensor("w1", [nk2, P, nk1, P], dt_mm,
                            kind="ExternalInput")
    else:
        w1 = nc.dram_tensor("w1", [din, hid], dt_mm, kind="ExternalInput")
    b1c = nc.dram_tensor("b1c", [P, nk2], F32, kind="ExternalInput")
    w2 = nc.dram_tensor("w2", [hid, c_dim], dt_mm, kind="ExternalInput")
    # b2 replicated across partitions on host: added at PSUM eviction on DVE
    # (cheaper than a K=1 bias matmul on PE, and exact fp32 — no f32r
    # truncation of the bias).
    b2rep = nc.dram_tensor("b2rep", [P, c_dim], F32, kind="ExternalInput")
    comb = nc.dram_tensor("comb", [P, ncolt], F32, kind="ExternalInput")
    out = nc.dram_tensor("out", [cap, c_dim], F32, kind="ExternalOutput")

    # A ragged tail of < ~30 tokens past the last full 128-token tile would
    # still stream full 32x1000-row L2 tiles (+5% PE here). Those tokens run
    # class-major with swapped operands (W2 stationary, hT tail moving ->
    # `tail` rows per matmul) inside the main c0/j loop, reusing the live W2
    # slice and the PSUM bank the shortened main loop frees; bias+comb for
    # them are applied on the host.
    tail = cap % P
    tail = tail if 0 < tail <= 30 else 0
    tail_out = None
    if tail:
        tail_out = nc.dram_tensor("tail_out", [c_dim, tail], F32,
                                  kind="ExternalOutput")

    with tile.TileContext(nc) as tc:
        with (
            tc.tile_pool(name="wpool", bufs=1) as wpool,
            tc.tile_pool(name="w1s", bufs=w1_bufs) as w1s,
            tc.tile_pool(name="w2s", bufs=w2_bufs) as w2s,
            tc.tile_pool(name="cpool", bufs=1) as cpool,
            tc.tile_pool(name="xpool", bufs=x_bufs) as xpool,
            tc.tile_pool(name="hpool", bufs=1) as hpool,
            tc.tile_pool(name="opool", bufs=4) as opool,
            tc.tile_pool(name="ps1", bufs=ps1_bufs, space="PSUM") as ps1,
            tc.tile_pool(name="ps2", bufs=1, space="PSUM") as ps2,
        ):
            # ---- resident weights (bf16 only) / constants ----
            if not stream_w:
                w1_sb, w2_sb = [], []
                for k in range(nk1):
                    t = wpool.tile([P, hid], dt_mm, tag=f"w1_{k}",
                                   name=f"w1sb{k}")
                    nc.sync.dma_start(t[:], w1[k * P:(k + 1) * P, :])
                    w1_sb.append(t)
                for k in range(nk2):
                    t = wpool.tile([P, c_dim], dt_mm, tag=f"w2_{k}",
                                   name=f"w2sb{k}")
                    nc.sync.dma_start(t[:], w2[k * P:(k + 1) * P, :])
                    w2_sb.append(t)
            b1_sb = cpool.tile([P, nk2], F32, tag="b1")
            nc.sync.dma_start(b1_sb[:], b1c[:])
            comb_sb = cpool.tile([P, ncolt], F32, tag="comb")
            nc.sync.dma_start(comb_sb[:], comb[:])
            b2_sb = cpool.tile([P, c_dim], F32, tag="b2")
            nc.sync.dma_start(b2_sb[:], b2rep[:])

            for _ in range(repeats):
                for t0 in range(0, cap, tok_blk):
                    tb = min(tok_blk, cap - t0)
                    # ---- stream this token block (transposed) ----
                    # The j=0 weight block is issued before the xT tiles so
                    # the first matmul is not queued behind the whole block.
                    w1_first = None
                    if stream_w:
                        w1_first = w1s.tile([P, nk1, P], dt_mm, tag="w1s",
                                            name="w1t")
                        nc.sync.dma_start(w1_first[:], w1[0])
                    xt = []
                    for k in range(nk1):
                        t = xpool.tile([P, tok_blk], dt_mm, tag=f"xt_{k}",
                                       name=f"xt{k}")
                        nc.sync.dma_start(
                            t[:, :tb], xT[k * P:(k + 1) * P, t0:t0 + tb])
                        xt.append(t)
                    # ---- layer 1: hT tiles [128 hid, tb tok] ----
                    # PSUM tiles are <=512 f32 wide, so each j group runs in
                    # sub-blocks sharing one W1 column-block load. Sub sizes
                    # are balanced (float32r matmuls drop to 1/4 rate below
                    # 256 moving rows, so avoid e.g. (512, 134) tails) and
                    # even (fp32r ISA requires even moving/dst sizes).
                    n_sub = max(1, -(-tb // 512))
                    base2, extra2 = divmod(tb // 2, n_sub)
                    subs, s0 = [], 0
                    for i in range(n_sub):
                        sw = 2 * (base2 + (1 if i < extra2 else 0))
                        if i == n_sub - 1:
                            sw = tb - s0
                        subs.append((s0, sw))
                        s0 += sw
                    ht = []
                    for j in range(nk2):
                        if stream_w:
                            if j == 0:
                                w1t = w1_first
                            else:
                                w1t = w1s.tile([P, nk1, P], dt_mm, tag="w1s",
                                               name="w1t")
                                nc.sync.dma_start(w1t[:], w1[j])
                        h = hpool.tile([P, tok_blk], dt_mm, tag=f"ht_{j}",
                                       name=f"ht{j}")
                        for s0, sw in subs:
                            ps = ps1.tile([P, 512], F32, tag="ps1",
                                          name="ps1t")
                            for k in range(nk1):
                                lhsT = (w1t[:, k, :] if stream_w
                                        else w1_sb[k][:, j * P:(j + 1) * P])
                                nc.tensor.matmul(
                                    ps[:, :sw], lhsT, xt[k][:, s0:s0 + sw],
                                    start=(k == 0), stop=(k == nk1 - 1))
                            nc.scalar.activation(
                                h[:, s0:s0 + sw], ps[:, :sw],
                                mybir.ActivationFunctionType.Relu,
                                bias=b1_sb[:, j:j + 1])
                        ht.append(h)
                    # ---- layer 2: out tiles [tw tok, cw classes] ----
                    # c0 outer + j middle keeps only ceil(tb/P) PSUM groups
                    # live while W2 streams through a small slice pool.
                    blk_tail = tail if t0 + tb == cap else 0
                    tb_main = tb - blk_tail
                    n_ts = (tb_main + P - 1) // P
                    tail_stride = 8   # 32 B chunk stride in PSUM (8B align)
                    for c0 in range(0, c_dim, 512):
                        cw = min(512, c_dim - c0)
                        pss = [ps2.tile([P, 512], F32, tag=f"ps2_{i}",
                                        name=f"ps2t{i}") for i in range(n_ts)]
                        ps_tail = None
                        if blk_tail:
                            # class-major tail accumulator in the PSUM bank
                            # the shortened main loop leaves free
                            ps_tail = ps2.tile([P, 512], F32,
                                               tag=f"ps2_{n_ts}",
                                               name="ps2tail")
                            n_cc = (cw + P - 1) // P
                            # chunk order per j; at the closing j a full-128-
                            # row chunk goes last so every eviction can hang
                            # its dependency on the closing matmul's columns
                            cc_full = max(
                                c for c in range(n_cc)
                                if min(P, cw - c * P) == P)
                            cc_close = [c for c in range(n_cc)
                                        if c != cc_full] + [cc_full]
                        for j in range(nk2):
                            if stream_w:
                                w2j = w2s.tile([P, 512], dt_mm, tag="w2s",
                                               name="w2t")
                                nc.sync.dma_start(
                                    w2j[:, :cw],
                                    w2[j * P:(j + 1) * P, c0:c0 + cw])
                                w2j = w2j[:, :cw]
                            else:
                                w2j = w2_sb[j][:, c0:c0 + cw]
                            for i in range(n_ts):
                                ts = i * P
                                tw = min(P, tb - ts)
                                nc.tensor.matmul(
                                    pss[i][:tw, :cw],
                                    ht[j][:, ts:ts + tw],
                                    w2j,
                                    start=(j == 0), stop=(j == nk2 - 1))
                            if blk_tail:
                                # one PSUM group spans the whole bank: start
                                # clears the zero region once, later chunks
                                # overwrite-then-accumulate via has_written
                                order = (cc_close if j == nk2 - 1
                                         else range(n_cc))
                                for oi, cc in enumerate(order):
                                    ccw = min(P, cw - cc * P)
                                    mm = nc.tensor.matmul(
                                        ps_tail[:ccw,
                                                cc * tail_stride:
                                                cc * tail_stride + blk_tail],
                                        w2j[:, cc * P:cc * P + ccw],
                                        ht[j][:, tb_main:tb_main + blk_tail],
                                        start=(j == 0 and oi == 0),
                                        stop=(j == nk2 - 1 and
                                              oi == n_cc - 1))
                        for i in range(n_ts):
                            ts = i * P
                            tw = min(P, tb - ts)
                            col = (t0 + ts) // P
                            ot = opool.tile([P, 512], F32, tag="ot", name="ot")
                            nc.vector.tensor_tensor(
                                ot[:tw, :cw], pss[i][:tw, :cw],
                                b2_sb[:tw, c0:c0 + cw],
                                op=mybir.AluOpType.add)
                            nc.vector.tensor_scalar_mul(
                                ot[:tw, :cw], ot[:tw, :cw],
                                comb_sb[:tw, col:col + 1])
                            nc.sync.dma_start(
                                out[t0 + ts:t0 + ts + tw, c0:c0 + cw],
                                ot[:tw, :cw])
                        if blk_tail:
                            # evict the group-closing chunk first (its read
                            # waits for the closing matmul); the other
                            # chunks read that SBUF copy as a bypass in1 so
                            # they are transitively ordered after the close
                            # (tensor_tensor may read only one PSUM input)
                            cl0 = cc_full * tail_stride
                            ot_cl = opool.tile([P, 512], F32, tag="ot",
                                               name="ot")
                            nc.vector.tensor_copy(
                                ot_cl[:, :blk_tail],
                                ps_tail[:, cl0:cl0 + blk_tail])
                            r0 = c0 + cc_full * P
                            nc.sync.dma_start(
                                tail_out[r0:r0 + P, :],
                                ot_cl[:, :blk_tail])
                            for cc in range(n_cc):
                                if cc == cc_full:
                                    continue
                                ccw = min(P, cw - cc * P)
                                ot = opool.tile([P, 512], F32, tag="ot",
                                                name="ot")
                                nc.vector.tensor_tensor(
                                    ot[:ccw, :blk_tail],
                                    ps_tail[:ccw,
                                            cc * tail_stride:
                                            cc * tail_stride + blk_tail],
                                    ot_cl[:ccw, :blk_tail],
                                    op=mybir.AluOpType.bypass)
                                r0 = c0 + cc * P
                                nc.sync.dma_start(
                                    tail_out[r0:r0 + ccw, :],
                                    ot[:ccw, :blk_tail])
    nc.compile()
    return nc


def make_in_maps(x, W1, b1, W2, b2, idx_per_e, w_per_e, cap, dt_mm=F32R,
                 stream_w=True):
    """Per-core input dicts (this is the scatter half of the all-to-all)."""
    npdt = _np_dt(dt_mm)
    nk1, nk2 = DIN // P, HID // P
    ncolt = (cap + P - 1) // P
    in_maps = []
    for e in range(E):
        idx, wts = idx_per_e[e], w_per_e[e]
        n = len(idx)
        xT = np.zeros((DIN, cap), dtype=npdt)
        xT[:, :n] = x[idx].T.astype(npdt)
        flat = np.zeros(ncolt * P, dtype=np.float32)
        flat[:n] = wts
        combp = np.ascontiguousarray(flat.reshape(ncolt, P).T)
        w1e = W1[e].astype(npdt)
        if stream_w:
            w1e = np.ascontiguousarray(
                w1e.reshape(nk1, P, nk2, P).transpose(2, 1, 0, 3))
        in_maps.append({
            "xT": xT,
            "w1": w1e,
            "b1c": np.ascontiguousarray(b1[e].reshape(HID // P, P).T),
            "w2": W2[e].astype(npdt),
            "b2rep": np.ascontiguousarray(
                np.broadcast_to(b2[e].astype(np.float32), (P, C))),
            "comb": combp,
        })
    return in_maps


def combine_host(results, idx_per_e, w_per_e, b2, cap):
    tail = cap % P
    tail = tail if 0 < tail <= 30 else 0
    n_main_cap = cap - tail
    out = np.zeros((B, C), dtype=np.float32)
    for e in range(E):
        idx, wts = idx_per_e[e], w_per_e[e]
        n = len(idx)
        n_main = min(n, n_main_cap)
        if n_main:
            # idx is unique within one expert (an expert appears at most once
            # in a token's top-2), so fancy-index += is a safe scatter-add.
            out[idx[:n_main]] += results[e]["out"][:n_main]
        if n > n_main:
            # tokens from the swapped-operand tail path: raw h@W2 came back
            # class-major; apply bias and comb weight here.
            k = n - n_main
            y = results[e]["tail_out"][:, :k].T + b2[e]
            out[idx[n_main:n]] += wts[n_main:n, None] * y
    return out


def kernel(x, Wg, bg, W1, b1, W2, b2):
    x = np.asarray(x); Wg = np.asarray(Wg); bg = np.asarray(bg)
    W1 = np.asarray(W1); b1 = np.asarray(b1)
    W2 = np.asarray(W2); b2 = np.asarray(b2)

    idx_per_e, w_per_e, lb_loss, _, _ = route_host(x, Wg, bg)
    # fp32r ISA needs even matmul sizes; keep cap even so every sub-block is.
    cap = max(P, max(len(i) for i in idx_per_e))
    cap += cap % 2

    nc = build_program(cap, w1_bufs=3, w2_bufs=12)
    in_maps = make_in_maps(x, W1, b1, W2, b2, idx_per_e, w_per_e, cap)
    res = run_bass_kernel_spmd(nc, in_maps, list(range(NCORES)))
    out = combine_host(res.results, idx_per_e, w_per_e, b2, cap)
    return out, lb_loss
